# revision 27
# baseline (speedup 1.0000x reference)
"""Distributed Trainium2 kernel for nn_ALEError_23742579212666.

Computes: loss = 0.7 * masked_mean((target-pred)^2, target>0)
               + 0.3 * mean(sobel(target) - sobel(pred))

Math notes:
  * sobel is linear with symmetric padding, so
    mean(sobel(t) - sobel(p)) = mean(sobel(t-p)) and the column-sum of the
    separable stencil collapses: smoothing [1,2,1] contributes a factor 4
    per axis (B,C,H -> 4^3 = 64), the derivative [-1,0,1] along W has
    column weights [-2, 0, ..., 0, +2].  Hence
      mean(sobel(d)) = 128 * sum_rows(d[..., W-1] - d[..., 0]) / N.
  * masked sum of squares sum m*d^2 (m = t>0, d = t-p) is split by
    column halves so no engine exceeds the ~5us/tile DMA cadence:
      lo half: z = d*m on VectorE, sum z^2 via ScalarE Square+accum.
      hi half: d2 = d^2 on ScalarE, sum m*d2 as accumulated 128x128
               "diagonal" matmuls trace(m^T d2) on TensorE.
    mask count: ones-matmul column sums of m on TensorE.
  * the first row-tile is split 512+512+1024 and the last 1536+512 so
    the pipeline fills faster during the DMA ramp and drains faster at
    the end (mid-stream DMAs stay at 1MB — smaller SWDGE transfers pay
    a ~2us fixed cost).

Sharding: pure data parallel over batch, 4 images per core; per-core
partial sums (a few floats) are combined on the host (an on-device
all-reduce has a ~20us latency floor, ~half the whole kernel runtime).
"""

import sys

import numpy as np

if "/opt/trn_rl_repo" not in sys.path:
    sys.path.insert(0, "/opt/trn_rl_repo")

B, C, H, W = 32, 1, 512, 1024
NCORES = 8
BP = B // NCORES                 # batches per core
FOLD = 2                         # W-rows folded per SBUF row
RT = BP * C * H // FOLD          # 1024 DRAM rows per core (folded view)
TW = W * FOLD                    # 2048
P = 128                          # SBUF partitions
NT = RT // P                     # 8 row-tiles per tensor per core
NTOT = float(B * C * H * W)      # 16777216
ALPHA = 0.3

# work-list: (row_tile, col_start, col_end).  Mid-stream tiles are full
# (128,2048) — DMAs below ~1MB pay a ~2us SWDGE fixed cost — but the
# first and last tiles are halved so the pipeline fills/drains faster.
STEPS = [(0, 0, 512), (0, 512, W), (0, W, TW)]
STEPS += [(i, 0, TW) for i in range(1, NT - 1)]
STEPS += [(NT - 1, 0, 3 * W // 2), (NT - 1, 3 * W // 2, TW)]
NS = len(STEPS)                  # 11
LO = 512                         # columns of z handled on DVE per step

_CACHE = {}


def _build_nc():
    from concourse import bacc, mybir, tile

    f32 = mybir.dt.float32
    bf16 = mybir.dt.bfloat16
    Act = mybir.ActivationFunctionType
    Alu = mybir.AluOpType
    AxX = mybir.AxisListType.X

    # no collectives and no partition-dependent code: build a single-core
    # NEFF (each core runs an identical independent copy) — avoids the
    # multi-core sync setup in the boot preamble.
    nc = bacc.Bacc("TRN2", target_bir_lowering=False, debug=False,
                   num_devices=1, enable_partition_id=False)
    t_ext = nc.declare_dram_parameter("target", [RT, TW], f32, isOutput=False)
    p_ext = nc.declare_dram_parameter("pred", [RT, TW], f32, isOutput=False)
    id_ext = nc.declare_dram_parameter("ident", [P, P], bf16, isOutput=False)
    out_ext = nc.declare_dram_parameter("out", [1, 16], f32, isOutput=True)

    # boundary-column slot for g stats: slot = 2k + j for target,
    # 4 + (2k + j) for pred, with j = W-row in fold, k = first/last.
    def g_slot(is_pred, j, k):
        return (4 if is_pred else 0) + 2 * k + j

    with tile.TileContext(nc) as tc:
        with (
            tc.tile_pool(name="io", bufs=6) as io,
            tc.tile_pool(name="mid", bufs=4) as mid,
            tc.tile_pool(name="one", bufs=1) as one,
            tc.tile_pool(name="ps", bufs=1, space="PSUM") as ps,
        ):
            ones_b = one.tile([P, 1], bf16)
            nc.vector.memset(ones_b[:], 1.0)
            ones_f = one.tile([P, 1], f32)
            nc.vector.memset(ones_f[:], 1.0)
            ident = one.tile([P, P], bf16)
            nc.sync.dma_start(out=ident[:], in_=id_ext[:, :])
            a_st = one.tile([P, NS], f32)    # per-step sum z_lo^2 rows
            # boundary cols, layout (P, 8 slots, NS steps) flattened
            g_st = one.tile([P, 8 * NS], f32)
            nc.vector.memset(g_st[:], 0.0)

            psC = ps.tile([P, P], f32)       # accum m_hi^T d2_hi (diag)
            psN = ps.tile([1, 512], f32)     # mask-count column sums

            cnt_col = one.tile([P, 1], f32)  # last step's mask count
            n_cnt = 0                        # count-matmul counter
            n_dg = 0                         # diag-matmul counter
            # last step's count rides the DVE accumulator instead of PE,
            # so the psN PSUM reduce leaves the critical tail path.
            TOT_CNT = sum((ce - cs) // 512 for _, cs, ce in STEPS[:-1])
            TOT_DG = sum((ce - cs - min(LO, ce - cs)) // P
                         for _, cs, ce in STEPS)
            # boundary columns (global col -> j = W-row in fold, k = 0
            # for W-first / 1 for W-last)
            BCOLS = [(0, 0, 0), (W - 1, 0, 1), (W, 1, 0), (TW - 1, 1, 1)]

            for s, (i, cs, ce) in enumerate(STEPS):
                wdt = ce - cs
                lo = min(LO, wdt)
                tb = io.tile([P, wdt], bf16, tag="tb")
                pb = io.tile([P, wdt], bf16, tag="pb")
                # SWDGE cast-DMA: f32 HBM -> bf16 SBUF
                nc.gpsimd.dma_start(out=tb[:],
                                    in_=t_ext[P * i:P * (i + 1), cs:ce])
                nc.gpsimd.dma_start(out=pb[:],
                                    in_=p_ext[P * i:P * (i + 1), cs:ce])

                d = mid.tile([P, wdt], bf16, tag="d")
                m = mid.tile([P, wdt], bf16, tag="m")
                nc.vector.tensor_tensor(d[:], tb[:], pb[:], Alu.subtract)
                if s == NS - 1:
                    nc.vector.tensor_scalar(m[:], tb[:], 0.0, None,
                                            Alu.is_gt, op1=Alu.add,
                                            accum_out=cnt_col[:])
                else:
                    nc.vector.tensor_scalar(m[:], tb[:], 0.0, None,
                                            Alu.is_gt)

                # lo columns: z = d*m on DVE, sum z^2 on ACT
                z = mid.tile([P, lo], bf16, tag="z")
                sj = mid.tile([P, lo], bf16, tag="sj")
                nc.vector.tensor_tensor(z[:], d[:, 0:lo], m[:, 0:lo],
                                        Alu.mult)
                nc.scalar.activation(sj[:], z[:], Act.Square,
                                     accum_out=a_st[:, s:s + 1])
                # hi columns: d2 = d^2 on ACT, sum m*d2 via PE diag matmuls
                if wdt > lo:
                    d2 = mid.tile([P, wdt - lo], bf16, tag="d2")
                    nc.scalar.activation(d2[:], d[:, lo:wdt], Act.Square)
                    for k in range(0, wdt - lo, P):
                        nc.tensor.matmul(psC[:], m[:, lo + k:lo + k + P],
                                         d2[:, k:k + P],
                                         start=(n_dg == 0),
                                         stop=(n_dg == TOT_DG - 1))
                        n_dg += 1

                # boundary columns present in this step -> g stats
                for gc, j, k in BCOLS:
                    if not (cs <= gc < ce):
                        continue
                    c = gc - cs
                    st_slot = g_slot(False, j, k) * NS + s
                    sp_slot = g_slot(True, j, k) * NS + s
                    nc.vector.tensor_copy(g_st[0:P, st_slot:st_slot + 1],
                                          tb[:, c:c + 1])
                    nc.vector.tensor_copy(g_st[0:P, sp_slot:sp_slot + 1],
                                          pb[:, c:c + 1])

                if s < NS - 1:
                    for k in range(0, wdt, 512):
                        nc.tensor.matmul(psN[:], ones_b[:], m[:, k:k + 512],
                                         start=(n_cnt == 0),
                                         stop=(n_cnt == TOT_CNT - 1))
                        n_cnt += 1

            # ---- tail: fold everything to one 16-float vector ----
            dgC = one.tile([P, P], f32)
            nc.vector.tensor_tensor(dgC[:], psC[:], ident[:], Alu.mult)
            red = one.tile([P, 2], f32)
            nc.vector.tensor_reduce(red[:, 0:1], a_st[:], AxX, Alu.add)
            nc.vector.tensor_reduce(red[:, 1:2], dgC[:], AxX, Alu.add)

            # psN's accumulation group closes at the second-to-last step,
            # so this reduce overlaps the last step's compute.
            npart = one.tile([1, 1], f32)
            nc.vector.tensor_reduce(npart[:], psN[:], AxX, Alu.add)

            fin = one.tile([P, 10], f32)
            nc.vector.tensor_tensor(fin[:, 0:1], red[:, 0:1], red[:, 1:2],
                                    Alu.add)
            nc.vector.tensor_reduce(
                fin[:, 1:9],
                g_st[:, :].rearrange("p (s n) -> p s n", s=8),
                AxX, Alu.add)
            nc.vector.tensor_copy(fin[:, 9:10], cnt_col[:])

            ps_fin = ps.tile([1, 16], f32)
            nc.tensor.matmul(ps_fin[0:1, 0:10], ones_f[:], fin[:],
                             start=True, stop=True)

            outsb = one.tile([1, 16], f32)
            nc.vector.memset(outsb[:], 0.0)
            nc.vector.tensor_copy(outsb[0:1, 0:1], ps_fin[0:1, 0:1])
            nc.vector.tensor_tensor(outsb[0:1, 1:2], npart[:],
                                    ps_fin[0:1, 9:10], Alu.add)
            nc.vector.tensor_copy(outsb[0:1, 2:10], ps_fin[0:1, 1:9])
            nc.sync.dma_start(out=out_ext[:, :], in_=outsb[:])

    nc.compile()
    return nc


def get_nc():
    if "nc" not in _CACHE:
        _CACHE["nc"] = _build_nc()
    return _CACHE["nc"]


def make_in_maps(pred, target):
    import ml_dtypes

    pred = np.ascontiguousarray(np.asarray(pred, dtype=np.float32))
    target = np.ascontiguousarray(np.asarray(target, dtype=np.float32))
    ident_np = np.eye(P, dtype=ml_dtypes.bfloat16)
    in_maps = []
    for c in range(NCORES):
        in_maps.append({
            "pred": pred[c * BP:(c + 1) * BP].reshape(RT, TW),
            "target": target[c * BP:(c + 1) * BP].reshape(RT, TW),
            "ident": ident_np,
        })
    return in_maps


def combine(results):
    """results: list (per core) of {"out": (1,16) f32} -> scalar loss."""
    S = NV = TF = TL = PF = PL = 0.0
    for c in range(NCORES):
        v = np.asarray(results[c]["out"], dtype=np.float64).reshape(16)
        S += v[0]
        NV += v[1]
        TF += v[2] + v[3]
        TL += v[4] + v[5]
        PF += v[6] + v[7]
        PL += v[8] + v[9]
    G = (TL - PL) - (TF - PF)
    loss = (1.0 - ALPHA) * (S / NV) + ALPHA * 128.0 * G / NTOT
    return np.asarray(loss, dtype=np.float32)


def kernel(pred, target):
    from concourse.bass_utils import run_bass_kernel_spmd

    nc = get_nc()
    in_maps = make_in_maps(pred, target)
    res = run_bass_kernel_spmd(nc, in_maps, core_ids=list(range(NCORES)))
    return combine(res.results)
